# revision 12
# baseline (speedup 1.0000x reference)
"""Trainium2 Bass kernel for nn_Net_default_6493990551755 (binarized MLP).

Network: x[8192,784] -> fc1(6144) -> BN -> hardtanh -> sign -> fc2(6144) -> BN
         -> hardtanh -> sign -> fc3(6144) -> BN -> hardtanh -> fc4(10) -> log_softmax
with sign-binarized weights on fc1..fc3 (BNN, straight-through-estimator forward).

Strategy (8 NeuronCores, data-parallel over batch):
  * Each core computes 1024 batch rows. BatchNorm statistics are global over the
    8192 batch -> tiny AllReduce of per-feature sums (+sumsq for layer 3).
  * Everything on-chip is FEATURE-major ([feature_partition, batch_free]) so each
    layer's output feeds the next matmul's moving operand directly and BN
    reductions run along the free axis.
  * Layers 2/3 multiply +-1 by +-1: fp8 DoubleRow matmuls with fp32 PSUM
    accumulation are EXACT (integer sums < 2^24) and run at the fp8 peak.
  * Layer 1: x split into fp16 hi+lo planes (two accumulating matmuls) ~= fp32.
  * Biases b1..b3 cancel under BN mean subtraction; only sign(h-mean) is needed
    for layers 1/2, full BN+hardtanh only for layer 3.
  * Engine/queue balance: DMA descriptors cost ~0.6us each on the issuing
    queue, so transfers are batched and split between the two HWDGE queues
    (sync: x/w1/spills/sign loads; scalar: w2/w3 prep, layer weight loads).
    Weight signing runs as two DVE ops (is_ge, *2-1) on the otherwise-idle
    vector engine; the sign-vs-mean passes are split ACT/DVE.
  * Collective order: AllGather(w2) -> AllReduce(bn1) -> AllGather(w3) ->
    AllReduce(bn2) -> AllReduce(bn3), so stats never queue behind a big slab.
  * Tail: BN3 affine+hardtanh fused into 2 DVE ops per chunk; fc4 accumulates a
    [10, batch] PSUM with stationary w4 chunks, PE-transposes to batch-major,
    then log_softmax along the free axis.
"""

import math
import os
import sys
from dataclasses import dataclass

import numpy as np

for _p in ("/opt/trn_rl_repo",):
    if _p not in sys.path and os.path.isdir(_p):
        sys.path.insert(0, _p)

import concourse.bass as bass
import concourse.mybir as mybir
import concourse.tile as tile
from concourse import bacc

f32 = mybir.dt.float32
f16 = mybir.dt.float16
bf16 = mybir.dt.bfloat16
fp8 = mybir.dt.float8e4
AX = mybir.AxisListType
OP = mybir.AluOpType
AF = mybir.ActivationFunctionType
DR = mybir.MatmulPerfMode.DoubleRow

EPS = 1e-5


@dataclass(frozen=True)
class Cfg:
    ncores: int = 8
    B: int = 1024          # batch rows per core
    H: int = 6144          # hidden width
    DIN: int = 784         # input features
    C: int = 10            # classes

    @property
    def JC(self):          # hidden 128-chunks
        return self.H // 128

    @property
    def JG(self):          # j groups == cores (weight-share groups)
        return self.ncores

    @property
    def JSH(self):         # hidden rows signed per core (w2/w3 shard)
        return self.H // self.ncores

    @property
    def JL(self):          # 128-chunks per j group
        return self.JSH // 128

    @property
    def KC1(self):         # input-feature 128-chunks (padded)
        return math.ceil(self.DIN / 128)

    @property
    def K1P(self):
        return self.KC1 * 128

    @property
    def KP2(self):         # hidden k-pairs for DoubleRow
        return self.JC // 2

    @property
    def NBS(self):         # batch free-dim chunk for matmuls
        return min(512, self.B)

    @property
    def NB(self):
        return self.B // self.NBS

    @property
    def B8(self):          # 128-row output chunks
        return self.B // 128


def build_body(tc, cfg: Cfg, io):
    nc = tc.nc
    JC, JG, JSH, JL = cfg.JC, cfg.JG, cfg.JSH, cfg.JL
    KC1, K1P, KP2 = cfg.KC1, cfg.K1P, cfg.KP2
    B, NBS, NB, B8, C, DIN, H = cfg.B, cfg.NBS, cfg.NB, cfg.B8, cfg.C, cfg.DIN, cfg.H
    NTOT = float(cfg.ncores * B)
    groups = [list(range(cfg.ncores))]
    SH = "Shared" if (cfg.ncores > 4 and not os.environ.get("BNN_FORCE_LOCAL")) else "Local"

    def dve_sign(out_ap, in_ap):
        """sign via two DVE ops (exact 0 input maps to +1, not 0 -- measure
        zero for continuous weights).  out = 2*(in >= 0) - 1."""
        nc.vector.tensor_scalar(
            out=out_ap, in0=in_ap, scalar1=0.0, scalar2=None, op0=OP.is_ge)
        nc.vector.tensor_scalar(
            out=out_ap, in0=out_ap, scalar1=2.0, scalar2=-1.0,
            op0=OP.mult, op1=OP.add)

    with (
        tc.tile_pool(name="dram", bufs=1, space="DRAM") as dpool,
        tc.tile_pool(name="per", bufs=1) as per,          # persistent sbuf
        tc.tile_pool(name="stage", bufs=4) as st,         # streaming sbuf tiles
        tc.tile_pool(name="psum", bufs=6, space="PSUM") as pp,
        tc.tile_pool(name="psum4", bufs=2, space="PSUM") as pp4,
    ):
        # ------------------------------------------------------------------
        # DRAM scratch
        # ------------------------------------------------------------------
        sgn1 = dpool.tile([H, K1P], f16, name="sgn1")          # full signed w1
        sgn2 = dpool.tile([JSH, H], bf16, name="sgn2")
        sgn3 = dpool.tile([JSH, H], bf16, name="sgn3")
        frag2 = dpool.tile([JC, 128, JSH], fp8, name="frag2")
        frag3 = dpool.tile([JC, 128, JSH], fp8, name="frag3")
        slab2 = dpool.tile([JG, JC, 128, JSH], fp8, name="slab2", addr_space=SH)
        slab3 = dpool.tile([JG, JC, 128, JSH], fp8, name="slab3", addr_space=SH)
        xhib = dpool.tile([B, K1P], f16, name="xhib")
        xlob = dpool.tile([B, K1P], f16, name="xlob")
        h1sp = dpool.tile([JG, NB, 128, JL, NBS], f32, name="h1sp")
        h2sp = dpool.tile([JG, NB, 128, JL, NBS], f16, name="h2sp")
        h3sp = dpool.tile([JG, NB, 128, JL, NBS], f16, name="h3sp")
        st1i = dpool.tile([128, JC], f32, name="st1i")
        st1o = dpool.tile([128, JC], f32, name="st1o", addr_space=SH)
        st2i = dpool.tile([128, JC], f32, name="st2i")
        st2o = dpool.tile([128, JC], f32, name="st2o", addr_space=SH)
        st3i = dpool.tile([128, 2, JC], f32, name="st3i")
        st3o = dpool.tile([128, 2, JC], f32, name="st3o", addr_space=SH)

        # ------------------------------------------------------------------
        # x -> transposed fp16 hi/lo planes (SBUF resident)   [sync queue]
        # ------------------------------------------------------------------
        for r in range(B // 128):
            xt = st.tile([128, DIN], f32, name="xt", bufs=2)
            nc.sync.dma_start(xt[:], io["x"][r * 128:(r + 1) * 128, :])
            xh = st.tile([128, K1P], f16, name="xh", bufs=2)
            xl = st.tile([128, K1P], f16, name="xl", bufs=2)
            if K1P > DIN:
                nc.vector.memset(xh[:, DIN:], 0.0)
                nc.vector.memset(xl[:, DIN:], 0.0)
            nc.vector.tensor_copy(xh[:, :DIN], xt[:])
            nc.vector.tensor_tensor(
                out=xl[:, :DIN], in0=xt[:], in1=xh[:, :DIN], op=OP.subtract)
            nc.sync.dma_start(xhib[r * 128:(r + 1) * 128, :], xh[:])
            nc.sync.dma_start(xlob[r * 128:(r + 1) * 128, :], xl[:])

        xthi, xtlo = [], []
        for t in range(KC1):
            th = per.tile([128, B], f16, name=f"xthi{t}")
            tl = per.tile([128, B], f16, name=f"xtlo{t}")
            nc.sync.dma_start_transpose(th[:], xhib[:, t * 128:(t + 1) * 128])
            nc.sync.dma_start_transpose(tl[:], xlob[:, t * 128:(t + 1) * 128])
            xthi.append(th)
            xtlo.append(tl)

        # ------------------------------------------------------------------
        # w1 prep: full local DVE sign -> bounce -> xbar transpose straight
        # into SBUF weight tiles, pipelined per jg     [sync queue]
        # ------------------------------------------------------------------
        w1gs = []
        for jg in range(JG):
            jrows = slice(jg * JSH, (jg + 1) * JSH)
            for r in range(JSH // 128):
                rows = slice(jg * JSH + r * 128, jg * JSH + (r + 1) * 128)
                wsg = st.tile([128, DIN], f32, name="wsg", bufs=2)
                nc.sync.dma_start(wsg[:], io["w1"][rows, :])
                sgo = st.tile([128, K1P], f16, name="sgo", bufs=2)
                if K1P > DIN:
                    nc.vector.memset(sgo[:, DIN:], 0.0)
                dve_sign(sgo[:, :DIN], wsg[:])
                nc.sync.dma_start(sgn1[rows, :], sgo[:])
            w1g = st.tile([128, KC1, JSH], f16, name="w1g", bufs=2)
            for t in range(KC1):
                nc.sync.dma_start_transpose(
                    w1g[:, t, :], sgn1[jrows, t * 128:(t + 1) * 128])
            w1gs.append(w1g)

        # ------------------------------------------------------------------
        # w2/w3 prep (sharded): batched loads, DVE sign, xbar transpose,
        # fp8 cast, paired frag writes.            [scalar queue]
        # ------------------------------------------------------------------
        KB = 1536

        def prep_weight(wsrc, sgn_b, frag_b):
            for r in range(JSH // 128):
                rows = slice(r * 128, (r + 1) * 128)
                for k0 in range(0, H, KB):
                    wt = st.tile([128, KB], f32, name="wt", bufs=2)
                    nc.scalar.dma_start(wt[:], wsrc[rows, k0:k0 + KB])
                    sg = st.tile([128, KB], bf16, name="sg", bufs=2)
                    dve_sign(sg[:], wt[:])
                    nc.scalar.dma_start(sgn_b[rows, k0:k0 + KB], sg[:])
            for t2 in range(JC // 2):
                t8 = st.tile([128, 2, JSH], fp8, name="t8", bufs=2)
                for s in range(2):
                    t = 2 * t2 + s
                    tt = st.tile([128, JSH], bf16, name="tt", bufs=2)
                    nc.scalar.dma_start_transpose(
                        tt[:], sgn_b[:, t * 128:(t + 1) * 128])
                    nc.vector.tensor_copy(t8[:, s, :], tt[:])
                nc.scalar.dma_start(frag_b[2 * t2:2 * t2 + 2], t8[:])

        prep_weight(io["w2s"], sgn2, frag2)
        nc.gpsimd.collective_compute(
            "AllGather", OP.bypass, replica_groups=groups,
            ins=[frag2.opt()], outs=[slab2.opt()])

        # fc4 weights/bias/identity prep (tiny)
        w4f = st.tile([128, JC, C], f32, name="w4f", bufs=1)
        nc.scalar.dma_start(
            w4f[:], io["w4t"].rearrange("(c p) n -> p c n", p=128))
        w4sb = per.tile([128, JC, C], f16, name="w4sb")
        nc.vector.tensor_copy(w4sb[:], w4f[:])
        b4c = per.tile([C, 1], f32, name="b4c")
        nc.scalar.dma_start(b4c[:], io["b4c"][:, :])
        idt = per.tile([C, C], f32, name="idt")
        nc.gpsimd.memset(idt[:], 1.0)
        nc.gpsimd.affine_select(
            out=idt[:], in_=idt[:], compare_op=OP.is_equal, fill=0.0,
            base=0, pattern=[[-1, C]], channel_multiplier=1)
        g3c = per.tile([128, JC], f32, name="g3c")
        nc.scalar.dma_start(g3c[:], io["g3c"][:, :])
        be3c = per.tile([128, JC], f32, name="be3c")
        nc.scalar.dma_start(be3c[:], io["be3c"][:, :])

        # ------------------------------------------------------------------
        # Layer 1: h1 = x @ sign(w1).T  (fp16 hi+lo), spill fp32 h1 to DRAM
        # ------------------------------------------------------------------
        stats1 = per.tile([128, JC * NB], f32, name="stats1")
        for jg in range(JG):
            w1g = w1gs[jg]
            for b in range(NB):
                for jl in range(JL):
                    jc = jg * JL + jl
                    ps = pp.tile([128, NBS], f32, name="mmt", tag="mmt")
                    for t in range(KC1):
                        lw = w1g[:, t, jl * 128:(jl + 1) * 128]
                        nc.tensor.matmul(
                            ps[:], lw, xthi[t][:, b * NBS:(b + 1) * NBS],
                            start=(t == 0), stop=False)
                        nc.tensor.matmul(
                            ps[:], lw, xtlo[t][:, b * NBS:(b + 1) * NBS],
                            start=False, stop=(t == KC1 - 1))
                    hst = st.tile([128, NBS], f32, name="hst", bufs=3)
                    nc.vector.tensor_scalar(
                        out=hst[:], in0=ps[:], scalar1=0.0, scalar2=None,
                        op0=OP.add, op1=OP.add,
                        accum_out=stats1[:, jc * NB + b:jc * NB + b + 1])
                    nc.sync.dma_start(h1sp[jg, b, :, jl, :], hst[:])

        # ------------------------------------------------------------------
        # w3 prep (AllGather deferred until after AR1)   [scalar queue]
        # ------------------------------------------------------------------
        prep_weight(io["w3s"], sgn3, frag3)

        def sign_pass(h_b, dt_in, s_out, negm, nm):
            """s_out[:, jc, :] = sign(h - mean); alternate ACT / DVE per jc."""
            for jg in range(JG):
                for jl in range(JL):
                    jc = jg * JL + jl
                    ld = st.tile([128, NB * NBS], dt_in, name=f"ld{nm}",
                                 bufs=(2 if dt_in == f32 else 3))
                    nc.sync.dma_start(
                        ld.rearrange("p (b n) -> p b n", b=NB),
                        h_b[jg, :, :, jl, :].rearrange("b p n -> p b n"))
                    if jc % 2 == 0:
                        nc.scalar.sign(
                            s_out[:, jc, :], ld[:], bias=negm[:, jc:jc + 1])
                    else:
                        g = st.tile([128, NB * NBS], f16, name=f"g{nm}", bufs=1)
                        nc.vector.tensor_scalar(
                            out=g[:], in0=ld[:], scalar1=negm[:, jc:jc + 1],
                            scalar2=0.0, op0=OP.add, op1=OP.is_ge)
                        nc.vector.tensor_scalar(
                            out=s_out[:, jc, :], in0=g[:],
                            scalar1=2.0, scalar2=-1.0, op0=OP.mult, op1=OP.add)

        # ------------------------------------------------------------------
        # BN1 mean AllReduce -> sign pass -> s1
        # ------------------------------------------------------------------
        sums1 = per.tile([128, JC], f32, name="sums1")
        nc.vector.reduce_sum(sums1[:], stats1.rearrange("p (j b) -> p j b", b=NB), axis=AX.X)
        nc.sync.dma_start(st1i[:], sums1[:])
        nc.gpsimd.collective_compute(
            "AllReduce", OP.add, replica_groups=groups,
            ins=[st1i.opt()], outs=[st1o.opt()])
        gsum1 = per.tile([128, JC], f32, name="gsum1")
        nc.sync.dma_start(gsum1[:], st1o[:])
        negm1 = per.tile([128, JC], f32, name="negm1")
        nc.vector.tensor_scalar_mul(negm1[:], gsum1[:], -1.0 / NTOT)

        s1 = per.tile([128, JC, B], fp8, name="s1", tag="sA")
        sign_pass(h1sp, f32, s1, negm1, "1")

        nc.gpsimd.collective_compute(
            "AllGather", OP.bypass, replica_groups=groups,
            ins=[frag3.opt()], outs=[slab3.opt()])

        # ------------------------------------------------------------------
        # Layers 2/3: fp8 DoubleRow matmuls (exact +-1 arithmetic)
        # ------------------------------------------------------------------
        def binary_layer(slab_b, s_in, hsp_b, stats, sq_stats):
            for jg in range(JG):
                for b in range(NB):
                    pss = [pp.tile([128, NBS], f32, name="mmt", tag="mmt")
                           for _ in range(JL)]
                    for t4 in range(KP2 // 2):
                        wt4 = st.tile([128, 4, JSH], fp8, name="wt4", bufs=2)
                        nc.scalar.dma_start(
                            wt4[:],
                            slab_b[jg, 4 * t4:4 * t4 + 4].rearrange("s p j -> p s j"))
                        for s in range(2):
                            t2 = 2 * t4 + s
                            rh = s_in[:, 2 * t2:2 * t2 + 2, b * NBS:(b + 1) * NBS]
                            for jl in range(JL):
                                nc.tensor.matmul(
                                    pss[jl][:],
                                    wt4[:, 2 * s:2 * s + 2, jl * 128:(jl + 1) * 128],
                                    rh, perf_mode=DR,
                                    start=(t2 == 0), stop=(t2 == KP2 - 1))
                    hsb = st.tile([128, JL, NBS], f16, name="hsb", bufs=1)
                    for jl in range(JL):
                        jc = jg * JL + jl
                        nc.vector.tensor_scalar(
                            out=hsb[:, jl, :], in0=pss[jl][:],
                            scalar1=0.0, scalar2=None, op0=OP.add, op1=OP.add,
                            accum_out=stats[:, jc * NB + b:jc * NB + b + 1])
                        if sq_stats is not None:
                            junk = st.tile([128, NBS], f32, name="junk", bufs=1)
                            nc.scalar.activation(
                                out=junk[:], in_=pss[jl][:], func=AF.Square,
                                accum_out=sq_stats[:, jc * NB + b:jc * NB + b + 1])
                    nc.sync.dma_start(hsp_b[jg, b], hsb[:])

        stats2 = per.tile([128, JC * NB], f32, name="stats2")
        binary_layer(slab2, s1, h2sp, stats2, None)

        sums2 = per.tile([128, JC], f32, name="sums2")
        nc.vector.reduce_sum(sums2[:], stats2.rearrange("p (j b) -> p j b", b=NB), axis=AX.X)
        nc.sync.dma_start(st2i[:], sums2[:])
        nc.gpsimd.collective_compute(
            "AllReduce", OP.add, replica_groups=groups,
            ins=[st2i.opt()], outs=[st2o.opt()])
        gsum2 = per.tile([128, JC], f32, name="gsum2")
        nc.sync.dma_start(gsum2[:], st2o[:])
        negm2 = per.tile([128, JC], f32, name="negm2")
        nc.vector.tensor_scalar_mul(negm2[:], gsum2[:], -1.0 / NTOT)

        s2 = per.tile([128, JC, B], fp8, name="s2", tag="sA")
        sign_pass(h2sp, f16, s2, negm2, "2")

        stats3 = per.tile([128, JC * NB], f32, name="stats3")
        sq3 = per.tile([128, JC * NB], f32, name="sq3")
        binary_layer(slab3, s2, h3sp, stats3, sq3)

        pk3 = per.tile([128, 2, JC], f32, name="pk3")
        nc.vector.reduce_sum(pk3[:, 0, :], stats3.rearrange("p (j b) -> p j b", b=NB), axis=AX.X)
        nc.vector.reduce_sum(pk3[:, 1, :], sq3.rearrange("p (j b) -> p j b", b=NB), axis=AX.X)
        nc.sync.dma_start(st3i[:], pk3[:])
        nc.gpsimd.collective_compute(
            "AllReduce", OP.add, replica_groups=groups,
            ins=[st3i.opt()], outs=[st3o.opt()])
        g3s = per.tile([128, 2, JC], f32, name="g3s")
        nc.sync.dma_start(g3s[:], st3o[:])

        # BN3 coefficients: y = A*h + Bc ; A = gamma*rsqrt(var+eps),
        # Bc = beta - mean*A
        m3 = per.tile([128, JC], f32, name="m3")
        nc.vector.tensor_scalar_mul(m3[:], g3s[:, 0, :], 1.0 / NTOT)
        ex2 = per.tile([128, JC], f32, name="ex2")
        nc.vector.tensor_scalar_mul(ex2[:], g3s[:, 1, :], 1.0 / NTOT)
        msq = per.tile([128, JC], f32, name="msq")
        nc.vector.tensor_tensor(out=msq[:], in0=m3[:], in1=m3[:], op=OP.mult)
        var3 = per.tile([128, JC], f32, name="var3")
        nc.vector.tensor_tensor(out=var3[:], in0=ex2[:], in1=msq[:], op=OP.subtract)
        sd3 = per.tile([128, JC], f32, name="sd3")
        epst = per.tile([128, 1], f32, name="epst")
        nc.vector.memset(epst[:], EPS)
        nc.scalar.activation(out=sd3[:], in_=var3[:], func=AF.Sqrt, bias=epst[:])
        rs3 = per.tile([128, JC], f32, name="rs3")
        nc.vector.reciprocal(rs3[:], sd3[:])
        A3 = per.tile([128, JC], f32, name="A3")
        nc.vector.tensor_tensor(out=A3[:], in0=rs3[:], in1=g3c[:], op=OP.mult)
        mA = per.tile([128, JC], f32, name="mA")
        nc.vector.tensor_tensor(out=mA[:], in0=m3[:], in1=A3[:], op=OP.mult)
        B3 = per.tile([128, JC], f32, name="B3")
        nc.vector.tensor_tensor(out=B3[:], in0=be3c[:], in1=mA[:], op=OP.subtract)

        # ------------------------------------------------------------------
        # Tail: y3 = hardtanh(BN3(h3)) per chunk; fc4 accumulates [10, B]
        # PSUM; PE-transpose to batch-major; log_softmax along free axis.
        # ------------------------------------------------------------------
        lgp = [pp4.tile([C, NBS], f32, name="lgp", tag="lgp") for _ in range(NB)]
        for jg in range(JG):
            for jl in range(JL):
                jc = jg * JL + jl
                ldh = st.tile([128, NB * NBS], f16, name="ldh3", bufs=2)
                nc.scalar.dma_start(
                    ldh.rearrange("p (b n) -> p b n", b=NB),
                    h3sp[jg, :, :, jl, :].rearrange("b p n -> p b n"))
                y3 = st.tile([128, NB * NBS], f16, name="y3", bufs=2)
                nc.vector.tensor_scalar(
                    out=y3[:], in0=ldh[:],
                    scalar1=A3[:, jc:jc + 1], scalar2=B3[:, jc:jc + 1],
                    op0=OP.mult, op1=OP.add)
                nc.vector.tensor_scalar(
                    out=y3[:], in0=y3[:], scalar1=1.0, scalar2=-1.0,
                    op0=OP.min, op1=OP.max)
                for b in range(NB):
                    nc.tensor.matmul(
                        lgp[b][:], w4sb[:, jc, :], y3[:, b * NBS:(b + 1) * NBS],
                        start=(jc == 0), stop=(jc == JC - 1))
        lgs = per.tile([C, B], f32, name="lgs")
        for b in range(NB):
            nc.vector.tensor_scalar(
                out=lgs[:, b * NBS:(b + 1) * NBS], in0=lgp[b][:],
                scalar1=b4c[:, 0:1], scalar2=None, op0=OP.add)
        for b8 in range(B8):
            bs = slice(b8 * 128, (b8 + 1) * 128)
            tp = pp.tile([128, C], f32, name="tp", tag="mmt")
            nc.tensor.transpose(tp[:], lgs[:, bs], idt[:])
            mx = st.tile([128, 1], f32, name="mx", bufs=2)
            nc.vector.reduce_max(mx[:], tp[:], axis=AX.X)
            nmx = st.tile([128, 1], f32, name="nmx", bufs=2)
            nc.vector.tensor_scalar_mul(nmx[:], mx[:], -1.0)
            ex = st.tile([128, C], f32, name="ex", bufs=2)
            se = st.tile([128, 1], f32, name="se", bufs=2)
            nc.scalar.activation(
                out=ex[:], in_=tp[:], func=AF.Exp, bias=nmx[:], accum_out=se[:])
            lse = st.tile([128, 1], f32, name="lse", bufs=2)
            nc.scalar.activation(out=lse[:], in_=se[:], func=AF.Ln)
            c1 = st.tile([128, 1], f32, name="c1", bufs=2)
            nc.vector.tensor_tensor(out=c1[:], in0=nmx[:], in1=lse[:], op=OP.subtract)
            ot = st.tile([128, C], f32, name="ot", bufs=2)
            nc.vector.tensor_scalar(
                out=ot[:], in0=tp[:], scalar1=c1[:], scalar2=None, op0=OP.add)
            nc.sync.dma_start(io["out0"][bs, :], ot[:])


def build_nc(cfg: Cfg):
    nc = bacc.Bacc(
        "TRN2", target_bir_lowering=False, debug=False,
        num_devices=cfg.ncores,
    )
    io = {}
    io["x"] = nc.dram_tensor("x", [cfg.B, cfg.DIN], f32, kind="ExternalInput").ap()
    io["w1"] = nc.dram_tensor("w1", [cfg.H, cfg.DIN], f32, kind="ExternalInput").ap()
    io["w2s"] = nc.dram_tensor("w2s", [cfg.JSH, cfg.H], f32, kind="ExternalInput").ap()
    io["w3s"] = nc.dram_tensor("w3s", [cfg.JSH, cfg.H], f32, kind="ExternalInput").ap()
    io["w4t"] = nc.dram_tensor("w4t", [cfg.H, cfg.C], f32, kind="ExternalInput").ap()
    io["b4c"] = nc.dram_tensor("b4c", [cfg.C, 1], f32, kind="ExternalInput").ap()
    io["g3c"] = nc.dram_tensor("g3c", [128, cfg.JC], f32, kind="ExternalInput").ap()
    io["be3c"] = nc.dram_tensor("be3c", [128, cfg.JC], f32, kind="ExternalInput").ap()
    io["out0"] = nc.dram_tensor("out0", [cfg.B, cfg.C], f32, kind="ExternalOutput").ap()

    with tile.TileContext(nc) as tc:
        build_body(tc, cfg, io)
    nc.compile()
    return nc


def make_in_maps(cfg: Cfg, x, w1, w2, w3, w4, b4, g3, be3):
    """Host-side glue: shard batch + weight rows, tiny layout rearrangements."""
    w1f = np.ascontiguousarray(w1)                        # full [H, DIN]
    w4t = np.ascontiguousarray(w4.T)                      # [H, C]
    b4c = np.ascontiguousarray(b4.reshape(cfg.C, 1))
    g3c = np.ascontiguousarray(g3.reshape(cfg.JC, 128).T)  # [128, JC]
    be3c = np.ascontiguousarray(be3.reshape(cfg.JC, 128).T)
    in_maps = []
    for c in range(cfg.ncores):
        in_maps.append(dict(
            x=np.ascontiguousarray(x[c * cfg.B:(c + 1) * cfg.B]),
            w1=w1f,
            w2s=np.ascontiguousarray(w2[c * cfg.JSH:(c + 1) * cfg.JSH]),
            w3s=np.ascontiguousarray(w3[c * cfg.JSH:(c + 1) * cfg.JSH]),
            w4t=w4t, b4c=b4c, g3c=g3c, be3c=be3c,
        ))
    return in_maps


_CACHED = {}


def _get_nc(cfg: Cfg):
    if cfg not in _CACHED:
        _CACHED[cfg] = build_nc(cfg)
    return _CACHED[cfg]


def kernel(**inputs) -> np.ndarray:
    cfg = Cfg()
    nc = _get_nc(cfg)
    in_maps = make_in_maps(
        cfg,
        np.asarray(inputs["x"], dtype=np.float32),
        np.asarray(inputs["w1"], dtype=np.float32),
        np.asarray(inputs["w2"], dtype=np.float32),
        np.asarray(inputs["w3"], dtype=np.float32),
        np.asarray(inputs["w4"], dtype=np.float32),
        np.asarray(inputs["b4"], dtype=np.float32),
        np.asarray(inputs["g3"], dtype=np.float32),
        np.asarray(inputs["be3"], dtype=np.float32),
    )
    from concourse.bass_utils import run_bass_kernel_spmd
    res = run_bass_kernel_spmd(nc, in_maps, core_ids=list(range(cfg.ncores)))
    out = np.concatenate([res.results[c]["out0"] for c in range(cfg.ncores)], axis=0)
    return np.ascontiguousarray(out.astype(np.float32))
